# revision 15
# baseline (speedup 1.0000x reference)
"""Distributed Trainium2 Bass kernel for CustomMultiheadAttention.

Problem (hardcoded): B=4, N=2048, D=1024, H=16, head_dim=64, f32 inputs.
    q/k/v = x @ W{q,k,v}.T ; attn = softmax(q k^T/8 + alibi, mask) ; out = (attn v) @ Wo.T

Sharding over 8 NeuronCores: 2 batch-groups x 4 head-groups.
Each core computes its 2 batches x 4 heads end-to-end and a partial
out-projection (row-sharded Wo); partials are summed on host.

Per-core schedule (v2): a single software-pipelined stream.  The kernel is
a balanced two-engine race: PE matmul work (~273us: proj 109 + scores 55 +
AV 109 + oproj 27) vs ScalarE exp work (~256us: 256 ACTIVATEs of N=1024).
The schedule keeps ScalarE dense by drip-feeding all projection and
out-projection matmuls into the attention blocks' PE slack instead of
running them as serial phases:

  - projections are emitted as ~1.7us units (one psum accumulation group)
    scheduled just-in-time before the attention block that needs them;
    v-proj units pace one tile ahead of the AV consumer inside the block.
  - attention blocks run (qc-major): (p0,b0) (p1,b0) (p0,b1) (p1,b1) per
    qc so each exp(alibi^T) tile set is loaded once and stays resident
    for exactly two nearby blocks.
  - out-projection for qc is popped from a deque inside later blocks'
    kc-loops (never a serial burst), results written as f16.
  - softmax normalize per block: both denominator rows -> one dd tile,
    one reciprocal, one gpsimd partition_broadcast, two DVE multiplies,
    one gpsimd partition-shift DMA for the odd head's aoT half.
  - ScalarE runs *only* the 256 exps; all PSUM->SBUF copies are DVE.
"""

import numpy as np
import ml_dtypes

B, N, D = 4, 2048, 1024
H, HD = 16, 64
SCALE = HD ** -0.5
NCORES = 8
BG, HG = 2, 4          # batch groups x head groups
B_LOC = B // BG        # 2 batches per core
H_LOC = H // HG        # 4 heads per core
NPAIR = H_LOC // 2     # 2 head pairs
F_LOC = H_LOC * HD     # 256 local features
DC = D // 128          # 8 contraction chunks for projections
TT = N // 128          # 16 token tiles
QC = N // 512          # 4 query chunks
KC = N // 128          # 16 key tiles
VW = H_LOC * (HD + 1)  # 260: v row width per token tile (65 per head)
KQRT = KC // 4         # kc chunks per ea quarter tile

BF16 = ml_dtypes.bfloat16
F16 = np.float16

_compiled = {}


def _build():
    import concourse.bass as bass
    import concourse.mybir as mybir
    import concourse.tile as tile
    from concourse import bacc
    from contextlib import ExitStack

    f32 = mybir.dt.float32
    f16 = mybir.dt.float16
    bf16 = mybir.dt.bfloat16
    EXP = mybir.ActivationFunctionType.Exp

    nc = bacc.Bacc()

    xT = nc.declare_dram_parameter("xT", [B_LOC, DC, 128, N], bf16, isOutput=False)
    wqT = nc.declare_dram_parameter("wqT", [128, DC * F_LOC], bf16, isOutput=False)
    wkT = nc.declare_dram_parameter("wkT", [128, DC * F_LOC], bf16, isOutput=False)
    wvT = nc.declare_dram_parameter("wvT", [128, DC * F_LOC], bf16, isOutput=False)
    woT = nc.declare_dram_parameter("woT", [128, NPAIR * D], bf16, isOutput=False)
    eaT = nc.declare_dram_parameter("eaT", [H_LOC, N, N], bf16, isOutput=False)
    out = nc.declare_dram_parameter("out", [B_LOC, N, D], f16, isOutput=True)

    with tile.TileContext(nc) as tc, ExitStack() as ctx:
        persist = ctx.enter_context(tc.tile_pool(name="persist", bufs=1))
        xpool = ctx.enter_context(tc.tile_pool(name="xpool", bufs=1))
        eapool = ctx.enter_context(tc.tile_pool(name="eapool", bufs=8))
        apool = ctx.enter_context(tc.tile_pool(name="apool", bufs=5))
        work = ctx.enter_context(tc.tile_pool(name="work", bufs=2))
        psum = ctx.enter_context(tc.tile_pool(name="psum", bufs=2, space="PSUM"))

        # ---- resident weights ----
        wq_sb = persist.tile([128, DC * F_LOC], bf16)
        wk_sb = persist.tile([128, DC * F_LOC], bf16)
        wv_sb = persist.tile([128, DC * F_LOC], bf16)
        wo_sb = persist.tile([128, NPAIR * D], bf16)
        nc.sync.dma_start(out=wk_sb, in_=wkT[:, :])

        # ---- persistent activations ----
        qT_sb = [[persist.tile([128, N], bf16, name=f"qT_{b}_{pr}")
                  for pr in range(NPAIR)] for b in range(B_LOC)]
        kT_sb = [[persist.tile([128, N], bf16, name=f"kT_{b}_{pr}")
                  for pr in range(NPAIR)] for b in range(B_LOC)]
        v_sb = [persist.tile([128, TT * VW], bf16, name=f"v_{b}")
                for b in range(B_LOC)]
        aoT_sb = [[persist.tile([128, N], bf16, name=f"aoT_{b}_{pr}")
                   for pr in range(NPAIR)] for b in range(B_LOC)]

        for b in range(B_LOC):
            ones_ap = v_sb[b].rearrange("p (t h c) -> p t h c", t=TT, h=H_LOC)[
                :, :, :, HD:HD + 1]
            nc.vector.memset(ones_ap, 1.0)

        # ---- x chunk DMAs: [128, 1024] per (b, dc, half), tag-rotated b0->b1 ----
        xc = {}

        def emit_xdma(b, h):
            """Load x(b) half h = chunks (dc=0..7, tok half h)."""
            for dc in range(DC):
                x_t = xpool.tile([128, 1024], bf16, tag=f"x{dc}_{h}",
                                 name=f"x_{b}_{dc}_{h}")
                nc.sync.dma_start(out=x_t,
                                  in_=xT[b, dc][:, h * 1024:(h + 1) * 1024])
                xc[(b, dc, h)] = x_t

        emit_xdma(0, 0)
        nc.sync.dma_start(out=wq_sb, in_=wqT[:, :])
        emit_xdma(0, 1)
        nc.sync.dma_start(out=wv_sb, in_=wvT[:, :])
        nc.sync.dma_start(out=wo_sb, in_=woT[:, :])

        # ---- ea quarter DMAs (gpsimd queue) ----
        # ea set (qi, pr) = 4 quarter tiles [128, KQRT*1024] covering all kc,
        # layout [p, (kc, hi, q)] per quarter; loaded once, used by the two
        # blocks (qi, pr, b=0) and (qi, pr, b=1).
        ea_tiles = {}

        def emit_ea(qi, pr):
            if (qi, pr) in ea_tiles:
                return
            quarts = []
            for quart in range(4):
                ea_t = eapool.tile([128, KQRT * 1024], bf16, tag="ea",
                                   name=f"ea_{qi}_{pr}_{quart}")
                for hi in range(2):
                    h = pr * 2 + hi
                    src = eaT[h].rearrange("(kc p) q -> p kc q", p=128)[
                        :, quart * KQRT:(quart + 1) * KQRT,
                        qi * 512:(qi + 1) * 512]
                    dst = ea_t.rearrange("p (kc i q) -> p kc i q",
                                         kc=KQRT, i=2)[:, :, hi, :]
                    nc.sync.dma_start(out=dst, in_=src)
                quarts.append(ea_t)
            ea_tiles[(qi, pr)] = quarts

        # ---- projection / out-projection units ----
        def emit_kq(w_sb, dst, b, pr, g):
            h, sub = g // 2, g % 2
            pq = psum.tile([128, 512], f32, tag="p1", name="pq")
            for dc in range(DC):
                nc.tensor.matmul(
                    pq,
                    lhsT=w_sb[:, dc * F_LOC + pr * 128:dc * F_LOC + (pr + 1) * 128],
                    rhs=xc[(b, dc, h)][:, sub * 512:(sub + 1) * 512],
                    start=(dc == 0), stop=(dc == DC - 1),
                )
            nc.vector.tensor_copy(out=dst[b][pr][:, g * 512:(g + 1) * 512], in_=pq)

        def emit_v(b, tt):
            h, sub = tt // 8, tt % 8
            pv = psum.tile([128, 512], f32, tag="p1", name="pv")
            for dc in range(DC):
                nc.tensor.matmul(
                    pv[:, 0:F_LOC],
                    lhsT=xc[(b, dc, h)][:, sub * 128:(sub + 1) * 128],
                    rhs=wv_sb[:, dc * F_LOC:(dc + 1) * F_LOC],
                    start=(dc == 0), stop=(dc == DC - 1),
                )
            vdst = v_sb[b].rearrange("p (t h c) -> p t h c", t=TT, h=H_LOC)[
                :, tt, :, 0:HD]
            nc.vector.tensor_copy(out=vdst, in_=pv[:, 0:F_LOC].rearrange(
                "p (h c) -> p h c", h=H_LOC))

        def emit_oproj(b, tt, oc):
            po = psum.tile([128, 512], f32, tag="p1", name="po")
            for pr in range(NPAIR):
                nc.tensor.matmul(
                    po,
                    lhsT=aoT_sb[b][pr][:, tt * 128:(tt + 1) * 128],
                    rhs=wo_sb[:, pr * D + oc * 512:pr * D + (oc + 1) * 512],
                    start=(pr == 0), stop=(pr == NPAIR - 1),
                )
            o_t = work.tile([128, 512], f16, tag="o_t", bufs=2, name="o_t")
            nc.vector.tensor_copy(out=o_t, in_=po)
            nc.sync.dma_start(
                out=out[b, tt * 128:(tt + 1) * 128, oc * 512:(oc + 1) * 512],
                in_=o_t)

        # ---- static filler schedule ----
        # Unit encoding: ('q'|'k', b, pr, g) | ('v', b, tt) | ('x', b, g)
        def run_unit(u):
            kind = u[0]
            if kind == 'q':
                emit_kq(wq_sb, qT_sb, u[1], u[2], u[3])
            elif kind == 'k':
                emit_kq(wk_sb, kT_sb, u[1], u[2], u[3])
            elif kind == 'v':
                emit_v(u[1], u[2])
            elif kind == 'x':
                emit_xdma(u[1], u[2])
            elif kind == 'o':
                emit_oproj(u[1], u[2], u[3])

        # prefix: minimum to start block (q0, p0, b0)
        for g in range(4):
            run_unit(('k', 0, 0, g))
        run_unit(('q', 0, 0, 0))
        for tt in range(4):
            run_unit(('v', 0, tt))

        # per-block filler lists (popped <=2 per kc iteration, in order).
        # 'x' units are free riders (DMA emission only).
        fillers = {
            (0, 0, 0): [('q', 0, 0, 1), ('q', 0, 1, 0), ('k', 0, 1, 0),
                        ('k', 0, 1, 1)],
            (0, 1, 0): [('k', 0, 1, 2), ('k', 0, 1, 3), ('q', 0, 0, 2),
                        ('q', 0, 1, 1), ('x', 1, 0), ('k', 1, 0, 0),
                        ('q', 0, 0, 3), ('q', 0, 1, 2), ('k', 1, 0, 1),
                        ('q', 0, 1, 3), ('x', 1, 1), ('q', 1, 0, 0),
                        ('v', 1, 0), ('v', 1, 1)],
            (0, 0, 1): [('k', 1, 0, 2), ('k', 1, 0, 3), ('q', 1, 0, 1),
                        ('k', 1, 1, 0), ('k', 1, 1, 1), ('q', 1, 1, 0)],
            (0, 1, 1): [('k', 1, 1, 2), ('k', 1, 1, 3), ('q', 1, 0, 2),
                        ('q', 1, 1, 1)],
            (1, 0, 0): [('q', 1, 0, 3), ('q', 1, 1, 2)],
            (1, 1, 0): [('q', 1, 1, 3)],
        }
        # v-pacing for each batch's first attention block: v(b, tt) must be
        # emitted before AV(kc=tt); give it a 2-kc lead.
        v_paced = {
            (0, 0, 0): {kc: ('v', 0, kc + 2) for kc in range(2, 14)},
            (0, 0, 1): {kc: ('v', 1, kc + 2) for kc in range(0, 14)},
        }

        oproj_deque = []

        # ---- attention blocks ----
        for qi in range(QC):
            for pr, b in ((0, 0), (1, 0), (0, 1), (1, 1)):
                blk = (qi, pr, b)
                # ea sets for this qc; for qi=0 these are fresh ring slots,
                # for qi>0 they were prefetched at the end of the prior qc's
                # b=1 blocks (prefetching earlier would rotate slots a
                # pending block still reads).
                emit_ea(qi, pr)
                if (pr, b) == (0, 0):
                    emit_ea(qi, 1)

                ea_q = ea_tiles[(qi, pr)]
                must = list(fillers.get(blk, []))
                paced = v_paced.get(blk, {})

                pav = [psum.tile([128, 512], f32, tag=f"pav{hi}", bufs=1,
                                 name=f"pav{hi}") for hi in range(2)]

                def scores(kc):
                    ps = psum.tile([128, 1024], f32, tag="ps", bufs=2,
                                   name="ps")
                    for hi in range(2):
                        nc.tensor.matmul(
                            ps[:, hi * 512:(hi + 1) * 512],
                            lhsT=kT_sb[b][pr][hi * 64:(hi + 1) * 64,
                                              kc * 128:(kc + 1) * 128],
                            rhs=qT_sb[b][pr][hi * 64:(hi + 1) * 64,
                                             qi * 512:(qi + 1) * 512],
                            start=True, stop=True,
                        )
                    return ps

                def pop_fillers(kc):
                    if kc in paced:
                        run_unit(paced[kc])
                    budget = 2
                    while budget > 0 and must:
                        u = must.pop(0)
                        run_unit(u)
                        if u[0] != 'x':
                            budget -= 1
                    while budget > 0 and oproj_deque:
                        run_unit(oproj_deque.pop(0))
                        budget -= 1

                ps_cur = scores(0)
                for kc in range(KC):
                    ea_slice = ea_q[kc // KQRT][
                        :, (kc % KQRT) * 1024:(kc % KQRT + 1) * 1024]
                    a_t = apool.tile([128, 1024], bf16, tag="a_t", name="a_t")
                    nc.scalar.activation(a_t, ps_cur, EXP)
                    # split the exp(alibi) multiplies between DVE and GpSimd:
                    # DVE is otherwise the most loaded queue
                    if kc % 2 == 1:
                        nc.gpsimd.tensor_mul(a_t, a_t, ea_slice)
                    else:
                        nc.vector.tensor_mul(a_t, a_t, ea_slice)
                    if kc + 1 < KC:
                        ps_cur = scores(kc + 1)
                    for hi in range(2):
                        h = pr * 2 + hi
                        nc.tensor.matmul(
                            pav[hi][0:65, :],
                            lhsT=v_sb[b][:, kc * VW + h * (HD + 1):
                                         kc * VW + (h + 1) * (HD + 1)],
                            rhs=a_t[:, hi * 512:(hi + 1) * 512],
                            start=(kc == 0), stop=(kc == KC - 1),
                        )
                    pop_fillers(kc)

                # ---- normalize: denominators live in row 64 of each pav ----
                dd = work.tile([1, 1024], f32, tag="dd", bufs=1, name="dd")
                rr = work.tile([1, 1024], f32, tag="rr", bufs=1, name="rr")
                rb = work.tile([64, 1024], f32, tag="rb", bufs=1, name="rb")
                for hi in range(2):
                    nc.vector.tensor_copy(out=dd[0:1, hi * 512:(hi + 1) * 512],
                                          in_=pav[hi][64:65, :])
                nc.vector.reciprocal_approx_fast(rr[0:1, :], dd[0:1, :])
                nc.gpsimd.partition_broadcast(rb[0:64, :], rr[0:1, :])
                qs = slice(qi * 512, (qi + 1) * 512)
                nc.vector.tensor_mul(aoT_sb[b][pr][0:64, qs],
                                     pav[0][0:64, :], rb[0:64, 0:512])
                tmp = work.tile([64, 512], bf16, tag="aotmp", bufs=2,
                                name="aotmp")
                nc.vector.tensor_mul(tmp[0:64, :], pav[1][0:64, :],
                                     rb[0:64, 512:1024])
                nc.gpsimd.dma_start(out=aoT_sb[b][pr][64:128, qs],
                                    in_=tmp[0:64, :])

                # prefetch next qc's ea set into the slots this (qi, pr)
                # set occupied — safe only now that both readers are emitted
                if b == 1 and qi + 1 < QC:
                    emit_ea(qi + 1, pr)

                # out-projection for (qi, b) becomes ready once both pr done
                if pr == 1:
                    for tt in range(qi * 4, (qi + 1) * 4):
                        for oc in range(2):
                            oproj_deque.append(('o', b, tt, oc))

        # drain remaining out-projection work
        while oproj_deque:
            run_unit(oproj_deque.pop(0))

    nc.finalize()
    return nc


def _get_graph():
    if "nc" not in _compiled:
        _compiled["nc"] = _build()
    return _compiled["nc"]


def _prep_in_maps(x, alibi_bias, Wq, Wk, Wv, Wo):
    """Host-side shard + reformat. Returns in_maps for cores 0..7."""
    wq_g, wk_g, wv_g, wo_g, ea_g = [], [], [], [], []

    def _chunked(wT, nchunk, width):
        # [K, width] -> [128, nchunk*width] with chunk-major free dim
        return np.ascontiguousarray(
            wT.reshape(nchunk, 128, width).transpose(1, 0, 2).reshape(
                128, nchunk * width)).astype(BF16)

    for gh in range(HG):
        fs = slice(gh * F_LOC, (gh + 1) * F_LOC)
        wq_g.append(_chunked((Wq[fs, :] * SCALE).T, DC, F_LOC))
        wk_g.append(_chunked(Wk[fs, :].T, DC, F_LOC))
        wv_g.append(_chunked(Wv[fs, :].T, DC, F_LOC))
        wo_g.append(_chunked(Wo[:, fs].T, NPAIR, D))
        al = alibi_bias[0, gh * H_LOC:(gh + 1) * H_LOC]  # [H_LOC, N(q), N(k)]
        ea_g.append(np.ascontiguousarray(
            np.exp(al).transpose(0, 2, 1)).astype(BF16))  # [h, k, q]

    xT_b = []
    for gb in range(BG):
        xs = x[gb * B_LOC:(gb + 1) * B_LOC]  # [B_LOC, N, D]
        xT_b.append(np.ascontiguousarray(xs.transpose(0, 2, 1)).astype(
            BF16).reshape(B_LOC, DC, 128, N))

    in_maps = []
    for c in range(NCORES):
        gb, gh = c // HG, c % HG
        in_maps.append({
            "xT": xT_b[gb], "wqT": wq_g[gh], "wkT": wk_g[gh],
            "wvT": wv_g[gh], "woT": wo_g[gh], "eaT": ea_g[gh],
        })
    return in_maps


def _numpy_reference(x, mask, alibi_bias, Wq, Wk, Wv, Wo):
    """Exact fallback for unexpected inputs (e.g. mask with zeros)."""
    q = (x @ Wq.T).reshape(B, N, H, HD).transpose(0, 2, 1, 3)
    k = (x @ Wk.T).reshape(B, N, H, HD).transpose(0, 2, 1, 3)
    v = (x @ Wv.T).reshape(B, N, H, HD).transpose(0, 2, 1, 3)
    attn = np.einsum("bhqd,bhkd->bhqk", q, k).astype(np.float32) * SCALE
    attn = attn + alibi_bias
    attn = np.where(mask == 0, np.finfo(np.float32).min, attn)
    attn = attn - attn.max(axis=-1, keepdims=True)
    e = np.exp(attn)
    attn = e / e.sum(axis=-1, keepdims=True)
    out = np.einsum("bhqk,bhkd->bhqd", attn, v)
    out = out.transpose(0, 2, 1, 3).reshape(B, N, D)
    return (out @ Wo.T).astype(np.float32)


def kernel(x, mask, alibi_bias, Wq, Wk, Wv, Wo, _trace=False):
    x = np.asarray(x, dtype=np.float32)
    mask = np.asarray(mask)
    alibi_bias = np.asarray(alibi_bias, dtype=np.float32)
    Wq, Wk, Wv, Wo = (np.asarray(w, dtype=np.float32) for w in (Wq, Wk, Wv, Wo))

    if not mask.all():
        return _numpy_reference(x, mask, alibi_bias, Wq, Wk, Wv, Wo)

    from concourse.bass_utils import run_bass_kernel_spmd

    nc = _get_graph()
    in_maps = _prep_in_maps(x, alibi_bias, Wq, Wk, Wv, Wo)
    res = run_bass_kernel_spmd(nc, in_maps, core_ids=list(range(NCORES)),
                               trace=_trace)
    full = np.zeros((B, N, D), dtype=np.float32)
    for c in range(NCORES):
        gb = c // HG
        full[gb * B_LOC:(gb + 1) * B_LOC] += res.results[c]["out"].astype(
            np.float32)
    if _trace:
        kernel.last_exec_time_ns = res.exec_time_ns
        kernel.last_results = res
    return full


# revision 22
# speedup vs baseline: 1.5862x; 1.5862x over previous
"""Distributed Trainium2 Bass kernel for CustomMultiheadAttention.

Problem (hardcoded): B=4, N=2048, D=1024, H=16, head_dim=64, f32 inputs.
    q/k/v = x @ W{q,k,v}.T ; attn = softmax(q k^T/8 + alibi, mask) ; out = (attn v) @ Wo.T

Sharding over 8 NeuronCores: 2 batch-groups x 4 head-groups.
Each core computes its 2 batches x 4 heads end-to-end and a partial
out-projection (row-sharded Wo); partials are summed on host.

Per-core schedule (v2): a single software-pipelined stream.  The kernel is
a balanced two-engine race: PE matmul work (~273us: proj 109 + scores 55 +
AV 109 + oproj 27) vs ScalarE exp work (~256us: 256 ACTIVATEs of N=1024).
The schedule keeps ScalarE dense by drip-feeding all projection and
out-projection matmuls into the attention blocks' PE slack instead of
running them as serial phases:

  - projections are emitted as ~1.7us units (one psum accumulation group)
    scheduled just-in-time before the attention block that needs them;
    v-proj units pace one tile ahead of the AV consumer inside the block.
  - attention blocks run (qc-major): (p0,b0) (p1,b0) (p0,b1) (p1,b1) per
    qc so each exp(alibi^T) tile set is loaded once and stays resident
    for exactly two nearby blocks.
  - out-projection for qc is popped from a deque inside later blocks'
    kc-loops (never a serial burst), results written as f16.
  - softmax normalize per block: both denominator rows -> one dd tile,
    one reciprocal, one gpsimd partition_broadcast, two DVE multiplies,
    one gpsimd partition-shift DMA for the odd head's aoT half.
  - ScalarE runs *only* the 256 exps; all PSUM->SBUF copies are DVE.
"""

import numpy as np
import ml_dtypes

B, N, D = 4, 2048, 1024
H, HD = 16, 64
SCALE = HD ** -0.5
NCORES = 8
BG, HG = 2, 4          # batch groups x head groups
B_LOC = B // BG        # 2 batches per core
H_LOC = H // HG        # 4 heads per core
NPAIR = H_LOC // 2     # 2 head pairs
F_LOC = H_LOC * HD     # 256 local features
DC = D // 128          # 8 contraction chunks for projections
TT = N // 128          # 16 token tiles
QC = N // 512          # 4 query chunks
KC = N // 128          # 16 key tiles
VW = H_LOC * (HD + 1)  # 260: v row width per token tile (65 per head)
KQRT = KC // 4         # kc chunks per ea quarter tile

BF16 = ml_dtypes.bfloat16
F16 = np.float16

_compiled = {}


def _build():
    import concourse.bass as bass
    import concourse.mybir as mybir
    import concourse.tile as tile
    from concourse import bacc
    from contextlib import ExitStack

    f32 = mybir.dt.float32
    f16 = mybir.dt.float16
    bf16 = mybir.dt.bfloat16
    EXP = mybir.ActivationFunctionType.Exp

    nc = bacc.Bacc()

    xT = nc.declare_dram_parameter("xT", [B_LOC, DC, 128, N], bf16, isOutput=False)
    wqT = nc.declare_dram_parameter("wqT", [128, DC * F_LOC], bf16, isOutput=False)
    wkT = nc.declare_dram_parameter("wkT", [128, DC * F_LOC], bf16, isOutput=False)
    wvT = nc.declare_dram_parameter("wvT", [128, DC * F_LOC], bf16, isOutput=False)
    woT = nc.declare_dram_parameter("woT", [128, NPAIR * D], bf16, isOutput=False)
    eaT = nc.declare_dram_parameter("eaT", [NPAIR, QC, 4, 128, KQRT * 1024],
                                    bf16, isOutput=False)
    out = nc.declare_dram_parameter("out", [B_LOC, N, D], f16, isOutput=True)

    with tile.TileContext(nc) as tc, ExitStack() as ctx:
        persist = ctx.enter_context(tc.tile_pool(name="persist", bufs=1))
        xpool = ctx.enter_context(tc.tile_pool(name="xpool", bufs=1))
        eapool = ctx.enter_context(tc.tile_pool(name="eapool", bufs=8))
        apool = ctx.enter_context(tc.tile_pool(name="apool", bufs=3))
        work = ctx.enter_context(tc.tile_pool(name="work", bufs=2))
        psum = ctx.enter_context(tc.tile_pool(name="psum", bufs=2, space="PSUM"))

        # ---- resident weights ----
        wq_sb = persist.tile([128, DC * F_LOC], bf16)
        wk_sb = persist.tile([128, DC * F_LOC], bf16)
        wv_sb = persist.tile([128, DC * F_LOC], bf16)
        wo_sb = persist.tile([128, NPAIR * D], bf16)
        nc.sync.dma_start(out=wk_sb, in_=wkT[:, :])

        # ---- persistent activations ----
        qT_sb = [[persist.tile([128, N], bf16, name=f"qT_{b}_{pr}")
                  for pr in range(NPAIR)] for b in range(B_LOC)]
        kT_sb = [[persist.tile([128, N], bf16, name=f"kT_{b}_{pr}")
                  for pr in range(NPAIR)] for b in range(B_LOC)]
        v_sb = [persist.tile([128, TT * VW], bf16, name=f"v_{b}")
                for b in range(B_LOC)]
        aoT_sb = [[persist.tile([128, N], bf16, name=f"aoT_{b}_{pr}")
                   for pr in range(NPAIR)] for b in range(B_LOC)]

        for b in range(B_LOC):
            ones_ap = v_sb[b].rearrange("p (t h c) -> p t h c", t=TT, h=H_LOC)[
                :, :, :, HD:HD + 1]
            nc.vector.memset(ones_ap, 1.0)

        # ---- x chunk DMAs: [128, 1024] per (b, dc, half), tag-rotated b0->b1 ----
        xc = {}

        def emit_xdma(b, h):
            """Load x(b) half h = chunks (dc=0..7, tok half h)."""
            for dc in range(DC):
                x_t = xpool.tile([128, 1024], bf16, tag=f"x{dc}_{h}",
                                 name=f"x_{b}_{dc}_{h}")
                nc.sync.dma_start(out=x_t,
                                  in_=xT[b, dc][:, h * 1024:(h + 1) * 1024])
                xc[(b, dc, h)] = x_t

        emit_xdma(0, 0)
        nc.sync.dma_start(out=wq_sb, in_=wqT[:, :])
        emit_xdma(0, 1)
        nc.sync.dma_start(out=wv_sb, in_=wvT[:, :])
        nc.sync.dma_start(out=wo_sb, in_=woT[:, :])

        # ---- ea quarter DMAs (gpsimd queue) ----
        # ea set (qi, pr) = 4 quarter tiles [128, KQRT*1024] covering all kc,
        # layout [p, (kc, hi, q)] per quarter; loaded once, used by the two
        # blocks (qi, pr, b=0) and (qi, pr, b=1).
        ea_tiles = {}

        def emit_ea(qi, pr):
            if (qi, pr) in ea_tiles:
                return
            quarts = []
            for quart in range(4):
                ea_t = eapool.tile([128, KQRT * 1024], bf16, tag="ea",
                                   name=f"ea_{qi}_{pr}_{quart}")
                nc.sync.dma_start(out=ea_t, in_=eaT[pr, qi, quart])
                quarts.append(ea_t)
            ea_tiles[(qi, pr)] = quarts

        # ---- projection / out-projection units ----
        def emit_kq(w_sb, dst, b, pr, g):
            h, sub = g // 2, g % 2
            pq = psum.tile([128, 512], f32, tag="p1", name="pq")
            for dc in range(DC):
                nc.tensor.matmul(
                    pq,
                    lhsT=w_sb[:, dc * F_LOC + pr * 128:dc * F_LOC + (pr + 1) * 128],
                    rhs=xc[(b, dc, h)][:, sub * 512:(sub + 1) * 512],
                    start=(dc == 0), stop=(dc == DC - 1),
                )
            nc.vector.tensor_copy(out=dst[b][pr][:, g * 512:(g + 1) * 512], in_=pq)

        def emit_v(b, tt):
            h, sub = tt // 8, tt % 8
            pv = psum.tile([128, 512], f32, tag="p1", name="pv")
            for dc in range(DC):
                nc.tensor.matmul(
                    pv[:, 0:F_LOC],
                    lhsT=xc[(b, dc, h)][:, sub * 128:(sub + 1) * 128],
                    rhs=wv_sb[:, dc * F_LOC:(dc + 1) * F_LOC],
                    start=(dc == 0), stop=(dc == DC - 1),
                )
            vdst = v_sb[b].rearrange("p (t h c) -> p t h c", t=TT, h=H_LOC)[
                :, tt, :, 0:HD]
            nc.vector.tensor_copy(out=vdst, in_=pv[:, 0:F_LOC].rearrange(
                "p (h c) -> p h c", h=H_LOC))

        def emit_oproj(b, tt, oc):
            po = psum.tile([128, 512], f32, tag="p1", name="po")
            for pr in range(NPAIR):
                nc.tensor.matmul(
                    po,
                    lhsT=aoT_sb[b][pr][:, tt * 128:(tt + 1) * 128],
                    rhs=wo_sb[:, pr * D + oc * 512:pr * D + (oc + 1) * 512],
                    start=(pr == 0), stop=(pr == NPAIR - 1),
                )
            o_t = work.tile([128, 512], f16, tag="o_t", bufs=2, name="o_t")
            nc.vector.tensor_copy(out=o_t, in_=po)
            nc.sync.dma_start(
                out=out[b, tt * 128:(tt + 1) * 128, oc * 512:(oc + 1) * 512],
                in_=o_t)

        # ---- static filler schedule ----
        # Unit encoding: ('q'|'k', b, pr, g) | ('v', b, tt) | ('x', b, g)
        def run_unit(u):
            kind = u[0]
            if kind == 'q':
                emit_kq(wq_sb, qT_sb, u[1], u[2], u[3])
            elif kind == 'k':
                emit_kq(wk_sb, kT_sb, u[1], u[2], u[3])
            elif kind == 'v':
                emit_v(u[1], u[2])
            elif kind == 'x':
                emit_xdma(u[1], u[2])
            elif kind == 'o':
                emit_oproj(u[1], u[2], u[3])

        # prefix: minimum to start block (q0, p0, b0)
        for g in range(4):
            run_unit(('k', 0, 0, g))
        run_unit(('q', 0, 0, 0))
        for tt in range(4):
            run_unit(('v', 0, tt))

        # per-block filler lists (popped <=2 per kc iteration, in order).
        # 'x' units are free riders (DMA emission only).
        fillers = {
            (0, 0, 0): [('q', 0, 0, 1), ('q', 0, 1, 0), ('k', 0, 1, 0),
                        ('k', 0, 1, 1)],
            (0, 1, 0): [('k', 0, 1, 2), ('k', 0, 1, 3), ('q', 0, 0, 2),
                        ('q', 0, 1, 1), ('x', 1, 0), ('k', 1, 0, 0),
                        ('q', 0, 0, 3), ('q', 0, 1, 2), ('k', 1, 0, 1),
                        ('q', 0, 1, 3), ('x', 1, 1), ('q', 1, 0, 0),
                        ('v', 1, 0), ('v', 1, 1)],
            (0, 0, 1): [('k', 1, 0, 2), ('k', 1, 0, 3), ('q', 1, 0, 1),
                        ('k', 1, 1, 0), ('k', 1, 1, 1), ('q', 1, 1, 0)],
            (0, 1, 1): [('k', 1, 1, 2), ('k', 1, 1, 3), ('q', 1, 0, 2),
                        ('q', 1, 1, 1)],
            (1, 0, 0): [('q', 1, 0, 3), ('q', 1, 1, 2)],
            (1, 1, 0): [('q', 1, 1, 3)],
        }
        # v-pacing for each batch's first attention block: v(b, tt) must be
        # emitted before AV(kc=tt); give it a one-pair lead.
        v_paced = {
            (0, 0, 0): {kp: [('v', 0, 2 * kp + 4), ('v', 0, 2 * kp + 5)]
                        for kp in range(0, 6)},
            (0, 0, 1): {kp: [('v', 1, 2 * kp + 2), ('v', 1, 2 * kp + 3)]
                        for kp in range(0, 7)},
        }

        oproj_deque = []

        # ---- attention blocks ----
        for qi in range(QC):
            for pr, b in ((0, 0), (1, 0), (0, 1), (1, 1)):
                blk = (qi, pr, b)
                # ea sets for this qc; for qi=0 these are fresh ring slots,
                # for qi>0 they were prefetched at the end of the prior qc's
                # b=1 blocks (prefetching earlier would rotate slots a
                # pending block still reads).
                emit_ea(qi, pr)
                if (pr, b) == (0, 0):
                    emit_ea(qi, 1)

                ea_q = ea_tiles[(qi, pr)]
                must = list(fillers.get(blk, []))
                paced = v_paced.get(blk, {})

                pav = [psum.tile([128, 512], f32, tag=f"pav{hi}", bufs=1,
                                 name=f"pav{hi}") for hi in range(2)]

                def scores(kc):
                    ps = psum.tile([128, 1024], f32, tag="ps", bufs=2,
                                   name="ps")
                    for hi in range(2):
                        nc.tensor.matmul(
                            ps[:, hi * 512:(hi + 1) * 512],
                            lhsT=kT_sb[b][pr][hi * 64:(hi + 1) * 64,
                                              kc * 128:(kc + 1) * 128],
                            rhs=qT_sb[b][pr][hi * 64:(hi + 1) * 64,
                                             qi * 512:(qi + 1) * 512],
                            start=True, stop=True,
                        )
                    return ps

                def pop_fillers(kp):
                    for u in paced.get(kp, ()):
                        run_unit(u)
                    budget = 2
                    while budget > 0 and must:
                        u = must.pop(0)
                        run_unit(u)
                        if u[0] != 'x':
                            budget -= 1
                    while budget > 0 and oproj_deque:
                        run_unit(oproj_deque.pop(0))
                        budget -= 1

                ps_cur = scores(0)
                ps_nxt = scores(1)
                for kp in range(KC // 2):
                    kc0, kc1 = 2 * kp, 2 * kp + 1
                    # one [128, 2048] tile holds exp(scores) for the kc pair;
                    # a single DVE multiply then applies exp(alibi) for both
                    a2 = apool.tile([128, 2048], bf16, tag="a_t", bufs=3,
                                    name="a2")
                    nc.scalar.activation(a2[:, 0:1024], ps_cur, EXP)
                    if kc0 + 2 < KC:
                        ps_cur = scores(kc0 + 2)
                    nc.scalar.activation(a2[:, 1024:2048], ps_nxt, EXP)
                    ea_slice = ea_q[kp // 2][
                        :, (kp % 2) * 2048:(kp % 2 + 1) * 2048]
                    nc.vector.tensor_mul(a2, a2, ea_slice)
                    if kc1 + 2 < KC:
                        ps_nxt = scores(kc1 + 2)
                    for kci, kc in ((0, kc0), (1, kc1)):
                        for hi in range(2):
                            h = pr * 2 + hi
                            nc.tensor.matmul(
                                pav[hi][0:65, :],
                                lhsT=v_sb[b][:, kc * VW + h * (HD + 1):
                                             kc * VW + (h + 1) * (HD + 1)],
                                rhs=a2[:, kci * 1024 + hi * 512:
                                       kci * 1024 + (hi + 1) * 512],
                                start=(kc == 0), stop=(kc == KC - 1),
                            )
                    pop_fillers(kp)

                # ---- normalize: denominators live in row 64 of each pav ----
                dd = work.tile([1, 1024], f32, tag="dd", bufs=1, name="dd")
                rr = work.tile([1, 1024], f32, tag="rr", bufs=1, name="rr")
                rb = work.tile([64, 1024], f32, tag="rb", bufs=1, name="rb")
                for hi in range(2):
                    nc.vector.tensor_copy(out=dd[0:1, hi * 512:(hi + 1) * 512],
                                          in_=pav[hi][64:65, :])
                nc.vector.reciprocal_approx_fast(rr[0:1, :], dd[0:1, :])
                nc.gpsimd.partition_broadcast(rb[0:64, :], rr[0:1, :])
                qs = slice(qi * 512, (qi + 1) * 512)
                nc.vector.tensor_mul(aoT_sb[b][pr][0:64, qs],
                                     pav[0][0:64, :], rb[0:64, 0:512])
                tmp = work.tile([64, 512], bf16, tag="aotmp", bufs=2,
                                name="aotmp")
                nc.vector.tensor_mul(tmp[0:64, :], pav[1][0:64, :],
                                     rb[0:64, 512:1024])
                nc.gpsimd.dma_start(out=aoT_sb[b][pr][64:128, qs],
                                    in_=tmp[0:64, :])

                # prefetch next qc's ea set into the slots this (qi, pr)
                # set occupied — safe only now that both readers are emitted
                if b == 1 and qi + 1 < QC:
                    emit_ea(qi + 1, pr)

                # out-projection for (qi, b) becomes ready once both pr done
                if pr == 1:
                    for tt in range(qi * 4, (qi + 1) * 4):
                        for oc in range(2):
                            oproj_deque.append(('o', b, tt, oc))

        # drain remaining out-projection work
        while oproj_deque:
            run_unit(oproj_deque.pop(0))

    nc.finalize()
    return nc


def _get_graph():
    if "nc" not in _compiled:
        _compiled["nc"] = _build()
    return _compiled["nc"]


def _prep_in_maps(x, alibi_bias, Wq, Wk, Wv, Wo):
    """Host-side shard + reformat. Returns in_maps for cores 0..7."""
    wq_g, wk_g, wv_g, wo_g, ea_g = [], [], [], [], []

    def _chunked(wT, nchunk, width):
        # [K, width] -> [128, nchunk*width] with chunk-major free dim
        return np.ascontiguousarray(
            wT.reshape(nchunk, 128, width).transpose(1, 0, 2).reshape(
                128, nchunk * width)).astype(BF16)

    for gh in range(HG):
        fs = slice(gh * F_LOC, (gh + 1) * F_LOC)
        wq_g.append(_chunked((Wq[fs, :] * SCALE).T, DC, F_LOC))
        wk_g.append(_chunked(Wk[fs, :].T, DC, F_LOC))
        wv_g.append(_chunked(Wv[fs, :].T, DC, F_LOC))
        wo_g.append(_chunked(Wo[:, fs].T, NPAIR, D))
        al = alibi_bias[0, gh * H_LOC:(gh + 1) * H_LOC]  # [H_LOC, N(q), N(k)]
        ea = np.exp(al).transpose(0, 2, 1).astype(BF16)  # [h, k, q]
        # on-chip layout: [pr, qc, quart, p, (kcq, hi, q)] so each quarter
        # tile is one fully contiguous DMA
        ea = ea.reshape(NPAIR, 2, 4, KQRT, 128, QC, 512)
        ea = ea.transpose(0, 5, 2, 4, 3, 1, 6)
        ea_g.append(np.ascontiguousarray(ea).reshape(
            NPAIR, QC, 4, 128, KQRT * 1024))

    xT_b = []
    for gb in range(BG):
        xs = x[gb * B_LOC:(gb + 1) * B_LOC]  # [B_LOC, N, D]
        xT_b.append(np.ascontiguousarray(xs.transpose(0, 2, 1)).astype(
            BF16).reshape(B_LOC, DC, 128, N))

    in_maps = []
    for c in range(NCORES):
        gb, gh = c // HG, c % HG
        in_maps.append({
            "xT": xT_b[gb], "wqT": wq_g[gh], "wkT": wk_g[gh],
            "wvT": wv_g[gh], "woT": wo_g[gh], "eaT": ea_g[gh],
        })
    return in_maps


def _numpy_reference(x, mask, alibi_bias, Wq, Wk, Wv, Wo):
    """Exact fallback for unexpected inputs (e.g. mask with zeros)."""
    q = (x @ Wq.T).reshape(B, N, H, HD).transpose(0, 2, 1, 3)
    k = (x @ Wk.T).reshape(B, N, H, HD).transpose(0, 2, 1, 3)
    v = (x @ Wv.T).reshape(B, N, H, HD).transpose(0, 2, 1, 3)
    attn = np.einsum("bhqd,bhkd->bhqk", q, k).astype(np.float32) * SCALE
    attn = attn + alibi_bias
    attn = np.where(mask == 0, np.finfo(np.float32).min, attn)
    attn = attn - attn.max(axis=-1, keepdims=True)
    e = np.exp(attn)
    attn = e / e.sum(axis=-1, keepdims=True)
    out = np.einsum("bhqk,bhkd->bhqd", attn, v)
    out = out.transpose(0, 2, 1, 3).reshape(B, N, D)
    return (out @ Wo.T).astype(np.float32)


def kernel(x, mask, alibi_bias, Wq, Wk, Wv, Wo, _trace=False):
    x = np.asarray(x, dtype=np.float32)
    mask = np.asarray(mask)
    alibi_bias = np.asarray(alibi_bias, dtype=np.float32)
    Wq, Wk, Wv, Wo = (np.asarray(w, dtype=np.float32) for w in (Wq, Wk, Wv, Wo))

    if not mask.all():
        return _numpy_reference(x, mask, alibi_bias, Wq, Wk, Wv, Wo)

    from concourse.bass_utils import run_bass_kernel_spmd

    nc = _get_graph()
    in_maps = _prep_in_maps(x, alibi_bias, Wq, Wk, Wv, Wo)
    res = run_bass_kernel_spmd(nc, in_maps, core_ids=list(range(NCORES)),
                               trace=_trace)
    full = np.zeros((B, N, D), dtype=np.float32)
    for c in range(NCORES):
        gb = c // HG
        full[gb * B_LOC:(gb + 1) * B_LOC] += res.results[c]["out"].astype(
            np.float32)
    if _trace:
        kernel.last_exec_time_ns = res.exec_time_ns
        kernel.last_results = res
    return full
